# revision 1
# baseline (speedup 1.0000x reference)
"""Fused single-head attention (QKV proj + softmax*scale + AV) on 8 trn2 cores.

Reference computation (fp32):
    qkv = x @ W.T            x:[4,4096,768]  W:[192,768]
    q,k,v = split(qkv, 64)
    A = q @ k.T              (no pre-softmax scale)
    out = softmax(A) / 8 @ v

Sharding: core c handles batch b=c//2, query half qh=c%2 (2048 queries),
full 4096 keys of that batch. SPMD-uniform program: the host rolls the
key/value columns of x^T by qh*2048 so every core's own queries are
always columns 0:2048 (softmax is permutation-invariant over keys).

Device dataflow (per core), matmuls in fp32r (1 cyc/col) or bf16:
    xt  [768, 4096] = x[b].T rolled
    wt  [768, 192]  = W[perm].T, perm = [K rows | V rows | Q rows]
    projection -> [K^T|V^T] (M=128) and Q^T (M=64) per 512-col block
    K^T stored pair-interleaved [128, 2048]: even k-tiles in rows 0:64,
    odd in rows 64:128; Q^T duplicated into both row halves. Scores for
    a k-tile pair run as two CONCURRENT row-group matmuls (contraction
    dh=64 needs only half the PE rows each).
    P^T = exp(A^T - 40) in bf16 (no row max needed: |A| <= ~77)
    out^T[dh,q] (+rowsum in row 64) accumulated in PSUM over k-tiles:
    lhsT = V_aug [128, 65] (col 64 = ones), rhs = P^T chunks.
    Finalize: out = PE-transpose(out^T) / (8 * rowsum).

Scheduling: the emission is software-pipelined -- AV of pair j is
emitted after the scores of pair j+1, so the PE never sits directly
behind the ACT exp; far-half projection and h0-finalize are interleaved
into the pair stream as PE filler.
"""

import sys

import numpy as np

for _p in ("/opt/trn_rl_repo",):
    if _p not in sys.path:
        sys.path.insert(0, _p)

import concourse.mybir as mybir  # noqa: E402
import concourse.tile as tile  # noqa: E402
from concourse import bacc  # noqa: E402
from concourse.bass_utils import run_bass_kernel_spmd  # noqa: E402
from concourse.masks import make_identity  # noqa: E402

B, S, D, DH = 4, 4096, 768, 64
QN = S // 2          # queries per core
NSB = 8              # 512-wide super-blocks of s
NKT = 32             # 128-wide key tiles
NPAIR = NKT // 2
HALF = 1024          # q-chunk for the main loop
EXP_BIAS = -40.0     # global score offset (softmax-invariant), fp32 headroom

F32 = mybir.dt.float32
F32R = mybir.dt.float32r
BF16 = mybir.dt.bfloat16

_NC_CACHE = None
LAST_RESULTS = None


def _build():
    nc = bacc.Bacc(num_devices=8)
    xt_d = nc.dram_tensor("xt", [D, S], F32R, kind="ExternalInput")
    # wt free cols: [K | V | Q | Qdup] -- Q duplicated so the M=128 lhsT
    # writes Q^T into both row halves in one matmul (saves a DVE copy)
    wt_d = nc.dram_tensor("wt", [128, 6, 4 * DH], F32R, kind="ExternalInput")
    out_d = nc.dram_tensor("out", [QN, DH], F32, kind="ExternalOutput")

    with tile.TileContext(nc) as tc:
        with (
            tc.tile_pool(name="big", bufs=1) as big,
            tc.tile_pool(name="psmm", bufs=3, space="PSUM") as psmm,
            tc.tile_pool(name="psacc", bufs=1, space="PSUM") as psacc,
            tc.tile_pool(name="pt", bufs=6) as ptp,
            tc.tile_pool(name="small", bufs=4) as small,
        ):
            xt_tiles = []
            for _sb in range(NSB):
                _xt = big.tile([128, 6, 512], F32R, tag=f"xt{_sb}")
                xt_tiles.append(_xt)
            wt_sb = big.tile([128, 6, 4 * DH], F32R)
            ktp = big.tile([128, NPAIR * 128], F32R)  # pair-interleaved K^T
            qt_sb = big.tile([128, QN], F32R)         # Q^T duplicated rows
            vt_sb = big.tile([64, S], BF16)
            v_sb = big.tile([128, NKT, 80], BF16)     # [...,0:64]=V, 64=ones
            acc_sb = big.tile([65, QN], F32)
            osb = big.tile([128, 16, DH], F32)
            ident = big.tile([128, 128], F32)
            identb = big.tile([128, 128], BF16)

            # warm-up source first so the PE can start immediately
            wsrc = big.tile([128, 512], BF16)
            nc.gpsimd.memset(wsrc[:], 0.0)
            make_identity(nc, ident[:])
            make_identity(nc, identb[:])
            nc.vector.memset(v_sb[:, :, 64:65], 1.0)
            ebias = big.tile([128, 1], F32)
            nc.vector.memset(ebias[:], EXP_BIAS)

            nc.sync.dma_start(out=wt_sb[:], in_=wt_d[:])

            # pre-issue all xt loads at chunk granularity: the sync queue
            # streams back-to-back (no dispatch-paced idle) while proj
            # matmuls still start on first-chunk arrival
            for _sb in range(NSB):
                _sl = slice(_sb * 512, (_sb + 1) * 512)
                _src = xt_d[:, _sl].rearrange("(k p) s -> p k s", p=128)
                for _kk in range(3):
                    nc.sync.dma_start(
                        out=xt_tiles[_sb][:, 2 * _kk:2 * _kk + 2, :],
                        in_=_src[:, 2 * _kk:2 * _kk + 2, :],
                    )

            # ~4us of zero-dependency dummy matmuls: opens the PE HAM
            # clock-gate (cold=1.2GHz, warm=2.4GHz) while input DMAs land
            for _w in range(16):
                wps = psmm.tile([128, HALF], F32, tag="mm")
                nc.tensor.matmul(
                    wps[:, 0:512], wsrc[:, 0:128], wsrc[:],
                    start=True, stop=True,
                )

            proj_state = {}

            def emit_proj_a(sb):
                sl = slice(sb * 512, (sb + 1) * 512)
                xt_t = xt_tiles[sb]
                kv_ps = psmm.tile([128, HALF], F32, tag="mm")
                for k in range(6):
                    nc.tensor.matmul(
                        kv_ps[:, 0:512],
                        wt_sb[:, k, 0:128],
                        xt_t[:, k, :],
                        start=(k == 0),
                        stop=(k == 5),
                    )
                # K^T into pair-interleaved layout: tile t = 4*sb+i
                for i in range(4):
                    t = 4 * sb + i
                    rh = 64 * (t % 2)
                    nc.vector.tensor_copy(
                        ktp[rh:rh + 64, (t // 2) * 128:(t // 2 + 1) * 128],
                        kv_ps[0:64, i * 128:(i + 1) * 128],
                    )
                nc.vector.tensor_copy(vt_sb[:, sl], kv_ps[64:128, 0:512])
                proj_state[sb] = kv_ps

            def emit_proj_b(sb):
                sl = slice(sb * 512, (sb + 1) * 512)
                xt_t = xt_tiles[sb]
                kv_ps = proj_state.pop(sb)
                if sb < 4:
                    q_ps = psmm.tile([128, HALF], F32, tag="mm")
                    for k in range(6):
                        nc.tensor.matmul(
                            q_ps[:, 0:512],
                            wt_sb[:, k, 128:256],
                            xt_t[:, k, :],
                            start=(k == 0),
                            stop=(k == 5),
                        )
                    nc.vector.tensor_copy(qt_sb[:, sl], q_ps[:, 0:512])
                # V natural tiles via PE transpose (bf16), 4 k-tiles per sb.
                # Target the unused second bank of kv_ps (bf16 view) so the
                # transposes never contend for a fresh PSUM slot.
                kv16 = kv_ps[:].bitcast(BF16)
                for t4 in range(4):
                    t = sb * 4 + t4
                    tsl = slice(1024 + 64 * t4, 1024 + 64 * (t4 + 1))
                    nc.tensor.transpose(
                        kv16[:, tsl],
                        vt_sb[:, t * 128:(t + 1) * 128],
                        identb[0:64, 0:64],
                    )
                    nc.vector.tensor_copy(v_sb[:, t, 0:64], kv16[:, tsl])

            def emit_proj(sb):
                emit_proj_a(sb)
                emit_proj_b(sb)

            ats = {}
            pts = {}
            accs = {}

            def emit_scores(h, j, split_exp=False):
                at_e = psmm.tile([128, HALF], F32, tag="mm")
                at_o = psmm.tile([128, HALF], F32, tag="mm")
                for g in range(2):
                    gsl = slice(h * HALF + g * 512, h * HALF + g * 512 + 512)
                    osl = slice(g * 512, (g + 1) * 512)
                    nc.tensor.matmul(
                        at_e[:, osl],
                        ktp[0:64, j * 128:(j + 1) * 128],
                        qt_sb[0:64, gsl],
                        start=True, stop=True,
                    )
                    nc.tensor.matmul(
                        at_o[:, osl],
                        ktp[64:128, j * 128:(j + 1) * 128],
                        qt_sb[64:128, gsl],
                        start=True, stop=True,
                    )
                pt_e = ptp.tile([128, HALF], BF16, tag="pt")
                pt_o = ptp.tile([128, HALF], BF16, tag="pt")
                if split_exp:
                    # per-512 halves: the g0 exps depend only on sb0's Q, so
                    # ACT starts ~5us earlier during the DMA-bound ramp
                    for g in range(2):
                        osl = slice(g * 512, (g + 1) * 512)
                        nc.scalar.activation(
                            out=pt_e[:, osl], in_=at_e[:, osl],
                            func=mybir.ActivationFunctionType.Exp, bias=ebias[:],
                        )
                        nc.scalar.activation(
                            out=pt_o[:, osl], in_=at_o[:, osl],
                            func=mybir.ActivationFunctionType.Exp, bias=ebias[:],
                        )
                else:
                    nc.scalar.activation(
                        out=pt_e[:], in_=at_e[:],
                        func=mybir.ActivationFunctionType.Exp, bias=ebias[:],
                    )
                    nc.scalar.activation(
                        out=pt_o[:], in_=at_o[:],
                        func=mybir.ActivationFunctionType.Exp, bias=ebias[:],
                    )
                pts[(h, j)] = (pt_e, pt_o)

            def emit_av(h, j):
                pt_e, pt_o = pts.pop((h, j))
                acc = accs[h]
                for tt, pt in ((2 * j, pt_e), (2 * j + 1, pt_o)):
                    for g in range(2):
                        osl = slice(g * 512, (g + 1) * 512)
                        nc.tensor.matmul(
                            acc[:, osl],
                            v_sb[:, tt, 0:65],
                            pt[:, osl],
                            start=(j == 0 and tt == 2 * j),
                            stop=(j == NPAIR - 1 and tt == 2 * j + 1),
                            skip_group_check=True,
                        )

            def emit_out_dma(oc):
                nc.sync.dma_start(
                    out=out_d[:].rearrange("(t p) d -> p t d", p=128)[
                        :, 4 * oc:4 * (oc + 1), :],
                    in_=osb[:, 4 * oc:4 * (oc + 1), :],
                )

            def emit_fin(gblk):
                ot = psmm.tile([128, HALF], F32, tag="mm")
                nc.tensor.transpose(
                    ot[:, 0:65],
                    acc_sb[:, gblk * 128:(gblk + 1) * 128],
                    ident[0:65, 0:65],
                )
                r = small.tile([128, 1], F32, tag="r")
                nc.vector.reciprocal(r[:], ot[:, 64:65])
                nc.vector.tensor_scalar(
                    osb[:, gblk, :], ot[:, 0:64], r[:], 0.125,
                    op0=mybir.AluOpType.mult, op1=mybir.AluOpType.mult,
                )

            # ---- emission schedule (software-pipelined) ----
            # h0 scores use q cols 0:1024 (projected from sb0+sb1), and pair
            # j's k-tiles come from sb j//2 -- so the main loop can start as
            # soon as two super-blocks have landed. Remaining projection is
            # interleaved one super-block every other pair, pacing the DMA.
            pairs = [(0, j) for j in range(NPAIR)] + [(1, j) for j in range(NPAIR)]
            acc_h0 = psacc.tile([65, HALF], F32, tag="acc")
            accs[0] = acc_h0

            proj_at = {0: 0, 1: 2, 2: 3, 4: 4, 6: 5, 8: 6, 10: 7}
            emitted_proj = 0
            def emit_ramp():
                # sb0 fully, then sb1 Q-first so scores(0,0) can issue early;
                # sb1's K/V parts follow after the first scores are emitted.
                emit_proj_a(0)
                emit_proj_b(0)
                sl1 = slice(512, 1024)
                xt1 = xt_tiles[1]
                for kk in range(3):
                    nc.sync.dma_start(
                        out=xt1[:, 2 * kk:2 * kk + 2, :],
                        in_=xt_d[:, sl1].rearrange(
                            "(k p) s -> p k s", p=128
                        )[:, 2 * kk:2 * kk + 2, :],
                    )
                q_ps = psmm.tile([128, HALF], F32, tag="mm")
                for k in range(6):
                    nc.tensor.matmul(
                        q_ps[:, 0:512],
                        wt_sb[:, k, 128:256],
                        xt1[:, k, :],
                        start=(k == 0),
                        stop=(k == 5),
                    )
                nc.vector.tensor_copy(qt_sb[:, sl1], q_ps[:, 0:512])

            def emit_ramp_rest():
                # the deferred K/V half of sb1 (K tiles 4..7 feed pairs 2,3)
                xt1 = xt_tiles[1]
                kv_ps = psmm.tile([128, HALF], F32, tag="mm")
                for k in range(6):
                    nc.tensor.matmul(
                        kv_ps[:, 0:512],
                        wt_sb[:, k, 0:128],
                        xt1[:, k, :],
                        start=(k == 0),
                        stop=(k == 5),
                    )
                for i2 in range(4):
                    t = 4 + i2
                    rh = 64 * (t % 2)
                    nc.vector.tensor_copy(
                        ktp[rh:rh + 64, (t // 2) * 128:(t // 2 + 1) * 128],
                        kv_ps[0:64, i2 * 128:(i2 + 1) * 128],
                    )
                nc.vector.tensor_copy(vt_sb[:, 512:1024], kv_ps[64:128, 0:512])
                kv16 = kv_ps[:].bitcast(BF16)
                for t4 in range(4):
                    t = 4 + t4
                    tsl = slice(1024 + 64 * t4, 1024 + 64 * (t4 + 1))
                    nc.tensor.transpose(
                        kv16[:, tsl],
                        vt_sb[:, t * 128:(t + 1) * 128],
                        identb[0:64, 0:64],
                    )
                    nc.vector.tensor_copy(v_sb[:, t, 0:64], kv16[:, tsl])

            for i, (h, j) in enumerate(pairs):
                if i in proj_at:
                    if i == 0:
                        emit_ramp()
                        emitted_proj += 2
                    else:
                        emit_proj_a(proj_at[i])
                        emitted_proj += 1
                emit_scores(h, j)
                if i == 0:
                    emit_ramp_rest()
                if i > 0:
                    ph, pj = pairs[i - 1]
                    emit_av(ph, pj)
                    if i in proj_at and i != 0:
                        emit_proj_b(proj_at[i])
                    if (ph, pj) == (0, NPAIR - 1):
                        # h0 accumulation complete: stage it and swap acc
                        nc.vector.tensor_copy(acc_sb[:, 0:512], accs[0][:, 0:512])
                        nc.vector.tensor_copy(acc_sb[:, 512:HALF], accs[0][:, 512:HALF])
                        acc_h1 = psacc.tile([65, HALF], F32, tag="acc")
                        accs[1] = acc_h1
                # h0 finalize (blocks 0..7) interleaved into h1 pair stream
                if h == 1 and j in (3, 5, 7, 9):
                    for blk in range(j - 3, j - 1):
                        emit_fin(blk)
                if h == 1 and j == 7:
                    emit_out_dma(0)
                if h == 1 and j == 11:
                    emit_out_dma(1)
            assert emitted_proj == NSB
            # final pair g-major so each 512-query half of the h1 acc is
            # staged to SBUF as soon as its last AV matmul retires
            ph, pj = pairs[-1]
            f_e, f_o = pts.pop((ph, pj))

            def _avmm(g, tt, pt):
                osl = slice(g * 512, (g + 1) * 512)
                nc.tensor.matmul(
                    accs[1][:, osl], v_sb[:, tt, 0:65], pt[:, osl],
                    start=False, stop=(tt == 2 * pj + 1),
                    skip_group_check=True,
                )

            _avmm(0, 2 * pj, f_e)
            _avmm(0, 2 * pj + 1, f_o)
            nc.vector.tensor_copy(acc_sb[:, HALF:HALF + 512], accs[1][:, 0:512])
            _avmm(1, 2 * pj, f_e)
            _avmm(1, 2 * pj + 1, f_o)
            nc.vector.tensor_copy(acc_sb[:, HALF + 512:2 * HALF],
                                  accs[1][:, 512:HALF])
            for gblk in range(8, 12):
                emit_fin(gblk)
            emit_out_dma(2)
            for gblk in range(12, 16):
                emit_fin(gblk)
            emit_out_dma(3)

    nc.finalize()
    return nc


def _get_nc():
    global _NC_CACHE
    if _NC_CACHE is None:
        _NC_CACHE = _build()
    return _NC_CACHE


def kernel(x, W, _trace=False):
    global LAST_RESULTS
    x = np.ascontiguousarray(np.asarray(x), dtype=np.float32)
    W = np.ascontiguousarray(np.asarray(W), dtype=np.float32)
    assert x.shape == (B, S, D) and W.shape == (3 * DH, D)

    # wt columns: [K | V | Q | Qdup] so proj M-tile0 = [K^T|V^T], and the
    # duplicated Q tile writes Q^T into both row halves in one matmul
    wtf = np.concatenate(
        [W[DH:2 * DH], W[2 * DH:], W[:DH], W[:DH]], axis=0
    ).T
    wt = np.ascontiguousarray(wtf.reshape(6, 128, 4 * DH).transpose(1, 0, 2))

    in_maps = []
    for c in range(8):
        b, qh = divmod(c, 2)
        xtb = x[b].T  # [768, 4096]
        if qh:
            xtc = np.ascontiguousarray(
                np.concatenate([xtb[:, QN:], xtb[:, :QN]], axis=1)
            )
        else:
            xtc = np.ascontiguousarray(xtb)
        in_maps.append({"xt": xtc, "wt": wt})

    nc = _get_nc()
    res = run_bass_kernel_spmd(nc, in_maps, list(range(8)), trace=_trace)
    LAST_RESULTS = res

    out = np.empty((B, S, DH), np.float32)
    for c in range(8):
        b, qh = divmod(c, 2)
        out[b, qh * QN:(qh + 1) * QN] = res.results[c]["out"]
    return out



# revision 7
# speedup vs baseline: 1.1541x; 1.1541x over previous
"""Fused single-head attention (QKV proj + softmax*scale + AV) on 8 trn2 cores.

Reference computation (fp32):
    qkv = x @ W.T            x:[4,4096,768]  W:[192,768]
    q,k,v = split(qkv, 64)
    A = q @ k.T              (no pre-softmax scale)
    out = softmax(A) / 8 @ v

Sharding: core c handles batch b=c//2, query half qh=c%2 (2048 queries),
full 4096 keys of that batch. SPMD-uniform program: the host rolls the
key/value columns of x^T by qh*2048 so every core's own queries are
always columns 0:2048 (softmax is permutation-invariant over keys).

v2 design notes (from perfetto trace of v1):
  - The PE serializes ALL matmuls (no row-group concurrency), so the
    kernel is PE-column-stream bound: proj 36864 + scores 65536 +
    AV 65536 cols at 1 col/cycle.
  - v1's fp32 input DMA (12.6 MB) starved the PE for ~25us; v2 ships
    x^T and W as fp16 (6.3 MB), which also passes precision easily
    (sim: 5e-3 rel err vs the 2e-2 gate; bf16 inputs FAIL at 2.7e-2).
  - Matmuls are emitted at the widest PSUM-legal width to amortize
    per-instruction overhead (~40-120 ns each).
  - exp stays on ACT (only engine with exp), bf16 out, constant bias
    -40 (softmax-invariant); P/V stay bf16 (fp16 lacks exp range,
    fp8 fails precision).

Per-core dataflow, 1024-wide super-blocks (NSB=4):
    proj: per sb a 6-chunk fp16 chain -> [K^T|V^T] and (sb<2) Q^T.
    kt [64,4096] f16, qt [64,2048] f16, vt bf16 -> PE-transposed into
    natural V tiles v_sb [128,t,0:64] (+ones col 64 for the rowsum).
    Slots (h,t): scores at[128k,1024q] = kt_t^T.qt_h -> exp -> P^T bf16
    -> AV acc[65,1024] += V_aug^T.P^T accumulated over t=0..31.
    Finalize: PE-transpose acc_sb 128-q blocks, out = ot/(8*rowsum).

Scheduling: chunk-granular input DMA; PE warms up on wt; proj chains,
V transposes and finalizes are interleaved into the slot stream as PE
filler; AV of slot i-1 is emitted after scores of slot i so the PE
never sits directly behind the ACT exp.
"""

import sys

import numpy as np

for _p in ("/opt/trn_rl_repo",):
    if _p not in sys.path:
        sys.path.insert(0, _p)

import concourse.mybir as mybir  # noqa: E402
import concourse.tile as tile  # noqa: E402
from concourse import bacc  # noqa: E402
from concourse.bass_utils import run_bass_kernel_spmd  # noqa: E402
from concourse.masks import make_identity  # noqa: E402

B, S, D, DH = 4, 4096, 768, 64
QN = S // 2          # queries per core
NSB = 4              # 1024-wide super-blocks of s
SBW = 1024
NKT = 32             # 128-wide key tiles
HALF = 1024          # q-half for the slot loop
EXP_BIAS = -40.0     # global score offset (softmax-invariant), fp32 headroom
MMW = 512            # matmul rhs width (hard ISA cap: N<=512 per matmul)

F32 = mybir.dt.float32
F16 = mybir.dt.float16
BF16 = mybir.dt.bfloat16

_NC_CACHE = None
LAST_RESULTS = None


def _build():
    nc = bacc.Bacc(num_devices=8)
    xt_d = nc.dram_tensor("xt", [D, S], F16, kind="ExternalInput")
    # wt cols: 6 contraction chunks x [K|V (128) | Q (64)]
    wt_d = nc.dram_tensor("wt", [128, 6 * 192], F16, kind="ExternalInput")
    out_d = nc.dram_tensor("out", [QN, DH], F32, kind="ExternalOutput")

    with tile.TileContext(nc) as tc:
        with (
            tc.tile_pool(name="big", bufs=1) as big,
            tc.tile_pool(name="psmm", bufs=3, space="PSUM") as psmm,
            tc.tile_pool(name="psacc", bufs=1, space="PSUM") as psacc,
            tc.tile_pool(name="pt", bufs=6) as ptp,
            tc.tile_pool(name="small", bufs=4) as small,
        ):
            xt_tiles = [
                big.tile([128, 6, SBW], F16, tag=f"xt{sb}", name=f"xt{sb}")
                for sb in range(NSB)
            ]
            wt_sb = big.tile([128, 6 * 192], F16)
            kt = big.tile([64, S], F16)
            qt = big.tile([64, QN], F16)
            vt = big.tile([64, S], BF16)
            v_sb = big.tile([128, NKT, 80], BF16)  # [...,0:64]=V, 64=ones
            acc_sb = big.tile([65, QN], F32)
            osb = big.tile([128, 16, DH], F32)
            ident = big.tile([128, 128], F32)
            identb = big.tile([128, 128], BF16)
            ebias = big.tile([128, 1], F32)
            escr = big.tile([128, 1], F32)

            # ---- input DMA split across issue queues: SP (hwdge) takes wt
            # + sb0 chunk-granular; ACT (hwdge) sb1; Pool (swdge on the idle
            # q7s) sb2+sb3. SP alone needs ~610ns per issue, serializing the
            # whole input stream ~15us; the split gets every sb moving early.
            def _xt_src(sb):
                return xt_d[:, sb * SBW:(sb + 1) * SBW].rearrange(
                    "(k p) s -> p k s", p=128
                )

            nc.sync.dma_start(out=wt_sb[:], in_=wt_d[:])
            for k in range(6):
                nc.sync.dma_start(
                    out=xt_tiles[0][:, k:k + 1, :], in_=_xt_src(0)[:, k:k + 1, :]
                )
            for k0 in range(0, 6, 2):
                nc.scalar.dma_start(
                    out=xt_tiles[1][:, k0:k0 + 2, :],
                    in_=_xt_src(1)[:, k0:k0 + 2, :],
                )
            for sb in (2, 3):
                for k0 in range(0, 6, 3):
                    nc.gpsimd.dma_start(
                        out=xt_tiles[sb][:, k0:k0 + 3, :],
                        in_=_xt_src(sb)[:, k0:k0 + 3, :],
                    )

            # ---- cheap setup off the PE
            wsrc = big.tile([128, 512], BF16)
            nc.vector.memset(wsrc[:], 0.0)
            nc.vector.memset(ebias[:], EXP_BIAS)
            nc.vector.memset(v_sb[:, :, 64:65], 1.0)
            make_identity(nc, ident[:])
            make_identity(nc, identb[:])
            # prime the ACT exp table set during the DMA window
            nc.scalar.activation(
                out=escr[:], in_=ebias[:],
                func=mybir.ActivationFunctionType.Exp, bias=ebias[:],
            )

            # ---- PE warmup, DMA-independent (opens the HAM clock-gate; a
            # ramp gap resets the 3us warm-up window, so bridge until sb0
            # chunks flow)
            for _w in range(6):
                wps = psmm.tile([128, HALF], F32, tag="mm")
                nc.tensor.matmul(
                    wps[:, 0:512], wsrc[:, 0:128], wsrc[:],
                    start=True, stop=True,
                )

            # ---- emission helpers --------------------------------------
            kv_state = {}

            def emit_kv(sb, k0, k1):
                """Chunks k0:k1 of sb's K/V projection chain."""
                if k0 == 0:
                    kv_state[sb] = psmm.tile(
                        [128, HALF], F32, tag="mm", name=f"kv_ps{sb}"
                    )
                ps = kv_state[sb]
                for k in range(k0, k1):
                    for c in range(HALF // MMW):
                        csl = slice(c * MMW, (c + 1) * MMW)
                        nc.tensor.matmul(
                            ps[:, csl],
                            wt_sb[:, k * 192:k * 192 + 128],
                            xt_tiles[sb][:, k, c * MMW:(c + 1) * MMW],
                            start=(k == 0), stop=(k == 5),
                        )
                if k1 == 6:
                    s0 = sb * SBW
                    nc.vector.tensor_copy(kt[:, s0:s0 + 512], ps[0:64, 0:512])
                    nc.vector.tensor_copy(
                        kt[:, s0 + 512:s0 + SBW], ps[0:64, 512:SBW]
                    )
                    nc.vector.tensor_copy(
                        vt[:, s0:s0 + 512], ps[64:128, 0:512]
                    )
                    nc.vector.tensor_copy(
                        vt[:, s0 + 512:s0 + SBW], ps[64:128, 512:SBW]
                    )

            def emit_q(sb, k0, k1):
                """Chunks k0:k1 of sb's Q projection chain (sb 0 or 1)."""
                if k0 == 0:
                    kv_state[("q", sb)] = psmm.tile(
                        [128, HALF], F32, tag="mm", name=f"q_ps{sb}"
                    )
                ps = kv_state[("q", sb)]
                for k in range(k0, k1):
                    for c in range(HALF // MMW):
                        csl = slice(c * MMW, (c + 1) * MMW)
                        nc.tensor.matmul(
                            ps[0:64, csl],
                            wt_sb[:, k * 192 + 128:(k + 1) * 192],
                            xt_tiles[sb][:, k, c * MMW:(c + 1) * MMW],
                            start=(k == 0), stop=(k == 5),
                        )
                if k1 == 6:
                    nc.vector.tensor_copy(
                        qt[:, sb * SBW:(sb + 1) * SBW], ps[0:64, :]
                    )
                    kv_state.pop(("q", sb))

            def emit_vtrans(sb, i0, i1):
                """V natural tiles i0:i1 (of 8) for sb via PE transpose into
                the spent kv_ps tile (bf16 view; WAR on the kt/vt copies)."""
                ps16 = kv_state[sb][:].bitcast(BF16)
                for i in range(i0, i1):
                    t = sb * 8 + i
                    tsl = slice(i * 64, (i + 1) * 64)
                    nc.tensor.transpose(
                        ps16[:, tsl],
                        vt[:, t * 128:(t + 1) * 128],
                        identb[0:64, 0:64],
                    )
                    nc.vector.tensor_copy(v_sb[:, t, 0:64], ps16[:, tsl])
                if i1 == 8:
                    kv_state.pop(sb)

            ats = {}
            pts = {}
            accs = {}

            def emit_scores(h, t):
                at = psmm.tile([128, HALF], F32, tag="mm")
                for c in range(HALF // MMW):
                    csl = slice(c * MMW, (c + 1) * MMW)
                    qsl = slice(h * HALF + c * MMW, h * HALF + (c + 1) * MMW)
                    nc.tensor.matmul(
                        at[:, csl],
                        kt[:, t * 128:(t + 1) * 128],
                        qt[:, qsl],
                        start=True, stop=True,
                    )
                pt = ptp.tile([128, HALF], BF16, tag="pt")
                nc.scalar.activation(
                    out=pt[:], in_=at[:],
                    func=mybir.ActivationFunctionType.Exp, bias=ebias[:],
                )
                pts[(h, t)] = pt

            def emit_av(h, t):
                pt = pts.pop((h, t))
                acc = accs[h]
                for c in range(HALF // MMW):
                    csl = slice(c * MMW, (c + 1) * MMW)
                    nc.tensor.matmul(
                        acc[:, csl],
                        v_sb[:, t, 0:65],
                        pt[:, csl],
                        start=(t == 0), stop=(t == NKT - 1),
                        skip_group_check=True,
                    )

            def emit_out_dma(oc):
                nc.sync.dma_start(
                    out=out_d[:].rearrange("(t p) d -> p t d", p=128)[
                        :, 4 * oc:4 * (oc + 1), :],
                    in_=osb[:, 4 * oc:4 * (oc + 1), :],
                )

            def emit_fin(gblks):
                """Batch: transpose each 128-q block into one psum tile, one
                batched reciprocal, then per-block scale (avoids the PE-DVE
                ping-pong that stalled the tail)."""
                n = len(gblks)
                ot = psmm.tile([128, HALF], F32, tag="mm")
                for j, g in enumerate(gblks):
                    nc.tensor.transpose(
                        ot[:, 128 * j:128 * j + 65],
                        acc_sb[:, g * 128:(g + 1) * 128],
                        ident[0:65, 0:65],
                    )
                r = small.tile([128, 4], F32, tag="r")
                nc.vector.reciprocal(
                    r[:, 0:n],
                    ot[:].rearrange("p (j c) -> p j c", c=128)[:, 0:n, 64:65],
                )
                for j, g in enumerate(gblks):
                    nc.vector.tensor_scalar(
                        osb[:, g, :], ot[:, 128 * j:128 * j + 64], r[:, j:j + 1],
                        0.125,
                        op0=mybir.AluOpType.mult, op1=mybir.AluOpType.mult,
                    )

            # ---- pre-slot ramp: sb0 proj (kv then q), then V tiles 0:8 ----
            emit_kv(0, 0, 6)
            emit_q(0, 0, 6)
            emit_vtrans(0, 0, 8)
            accs[0] = psacc.tile([65, HALF], F32, tag="acc", name="acc0")

            # ---- slot stream -------------------------------------------
            # filler[i]: PE work emitted right after scores of slot i.
            # kv_sb must land before its slots (kt t: sb1 -> slots 8.., etc);
            # Vtrans sb before its AV slots; q1 before slot 32; all proj
            # windows sit in h0 so h1 slots have a free psmm buf for fins.
            filler = {
                0: [("kv", 1, 0, 2)], 1: [("kv", 1, 2, 4)], 2: [("kv", 1, 4, 6)],
                3: [("vt", 1, 0, 2)], 4: [("vt", 1, 2, 4)],
                5: [("vt", 1, 4, 6)], 6: [("vt", 1, 6, 8)],
                7: [("kv", 2, 0, 2)], 8: [("kv", 2, 2, 4)], 9: [("kv", 2, 4, 6)],
                10: [("vt", 2, 0, 2)], 11: [("vt", 2, 2, 4)],
                12: [("vt", 2, 4, 6)], 13: [("vt", 2, 6, 8)],
                14: [("kv", 3, 0, 2)], 15: [("kv", 3, 2, 4)], 16: [("kv", 3, 4, 6)],
                17: [("vt", 3, 0, 2)], 18: [("vt", 3, 2, 4)],
                19: [("vt", 3, 4, 6)], 20: [("vt", 3, 6, 8)],
                21: [("q", 1, 0, 2)], 22: [("q", 1, 2, 4)], 23: [("q", 1, 4, 6)],
                # h1: finalize h0 blocks two per slot-pair; out DMAs staggered
                34: [("fin", (0, 1))], 36: [("fin", (2, 3))],
                38: [("fin", (4, 5))], 39: [("odma", 0)],
                40: [("fin", (6, 7))], 41: [("odma", 1)],
            }

            slots = [(0, t) for t in range(NKT)] + [(1, t) for t in range(NKT)]
            for i, (h, t) in enumerate(slots):
                emit_scores(h, t)
                for f in filler.get(i, ()):
                    if f[0] == "kv":
                        emit_kv(f[1], f[2], f[3])
                    elif f[0] == "q":
                        emit_q(f[1], f[2], f[3])
                    elif f[0] == "vt":
                        emit_vtrans(f[1], f[2], f[3])
                    elif f[0] == "fin":
                        emit_fin(f[1])
                    elif f[0] == "odma":
                        emit_out_dma(f[1])
                if i > 0:
                    ph, pt_ = slots[i - 1]
                    emit_av(ph, pt_)
                    if (ph, pt_) == (0, NKT - 1):
                        # h0 accumulation complete: stage and swap acc
                        nc.vector.tensor_copy(acc_sb[:, 0:512], accs[0][:, 0:512])
                        nc.vector.tensor_copy(
                            acc_sb[:, 512:HALF], accs[0][:, 512:HALF]
                        )
                        accs[1] = psacc.tile(
                            [65, HALF], F32, tag="acc", name="acc1"
                        )

            # ---- tail: last AV split so each 512-q chunk stages early ----
            fpt = pts.pop((1, NKT - 1))
            for c in range(2):
                csl = slice(c * 512, (c + 1) * 512)
                nc.tensor.matmul(
                    accs[1][:, csl], v_sb[:, NKT - 1, 0:65], fpt[:, csl],
                    start=False, stop=True, skip_group_check=True,
                )
                nc.vector.tensor_copy(
                    acc_sb[:, HALF + c * 512:HALF + (c + 1) * 512],
                    accs[1][:, csl],
                )
                emit_fin(tuple(range(8 + 4 * c, 12 + 4 * c)))
                emit_out_dma(2 + c)

    nc.finalize()
    return nc


def _get_nc():
    global _NC_CACHE
    if _NC_CACHE is None:
        _NC_CACHE = _build()
    return _NC_CACHE


def kernel(x, W, _trace=False):
    global LAST_RESULTS
    x = np.ascontiguousarray(np.asarray(x), dtype=np.float32)
    W = np.ascontiguousarray(np.asarray(W), dtype=np.float32)
    assert x.shape == (B, S, D) and W.shape == (3 * DH, D)

    # wt cols per contraction chunk k: [K rows | V rows | Q rows]
    wtf = np.concatenate([W[DH:2 * DH], W[2 * DH:], W[:DH]], axis=0).T
    wt = np.ascontiguousarray(
        wtf.reshape(6, 128, 192).transpose(1, 0, 2).reshape(128, 6 * 192)
    ).astype(np.float16)

    in_maps = []
    for c in range(8):
        b, qh = divmod(c, 2)
        xtb = x[b].T.astype(np.float16)  # [768, 4096]
        if qh:
            xtc = np.ascontiguousarray(
                np.concatenate([xtb[:, QN:], xtb[:, :QN]], axis=1)
            )
        else:
            xtc = np.ascontiguousarray(xtb)
        in_maps.append({"xt": xtc, "wt": wt})

    nc = _get_nc()
    res = run_bass_kernel_spmd(nc, in_maps, list(range(8)), trace=_trace)
    LAST_RESULTS = res

    out = np.empty((B, S, DH), np.float32)
    for c in range(8):
        b, qh = divmod(c, 2)
        out[b, qh * QN:(qh + 1) * QN] = res.results[c]["out"]
    return out


# revision 8
# speedup vs baseline: 1.2772x; 1.1066x over previous
"""Fused single-head attention (QKV proj + softmax*scale + AV) on 8 trn2 cores.

Reference computation (fp32):
    qkv = x @ W.T            x:[4,4096,768]  W:[192,768]
    q,k,v = split(qkv, 64)
    A = q @ k.T              (no pre-softmax scale)
    out = softmax(A) / 8 @ v

Sharding: core c handles batch b=c//2, query half qh=c%2 (2048 queries),
full 4096 keys of that batch. SPMD-uniform program: the host rolls the
key/value columns of x^T by qh*2048 so every core's own queries are
always columns 0:2048 (softmax is permutation-invariant over keys).

v2 design notes (from perfetto trace of v1):
  - The PE serializes ALL matmuls (no row-group concurrency), so the
    kernel is PE-column-stream bound: proj 36864 + scores 65536 +
    AV 65536 cols at 1 col/cycle.
  - v1's fp32 input DMA (12.6 MB) starved the PE for ~25us; v2 ships
    x^T and W as fp16 (6.3 MB), which also passes precision easily
    (sim: 5e-3 rel err vs the 2e-2 gate; bf16 inputs FAIL at 2.7e-2).
  - Matmuls are emitted at the widest PSUM-legal width to amortize
    per-instruction overhead (~40-120 ns each).
  - exp stays on ACT (only engine with exp), bf16 out, constant bias
    -40 (softmax-invariant); P/V stay bf16 (fp16 lacks exp range,
    fp8 fails precision).

Per-core dataflow, 1024-wide super-blocks (NSB=4):
    proj: per sb a 6-chunk fp16 chain -> [K^T|V^T] and (sb<2) Q^T.
    kt [64,4096] f16, qt [64,2048] f16, vt bf16 -> PE-transposed into
    natural V tiles v_sb [128,t,0:64] (+ones col 64 for the rowsum).
    Slots (h,t): scores at[128k,1024q] = kt_t^T.qt_h -> exp -> P^T bf16
    -> AV acc[65,1024] += V_aug^T.P^T accumulated over t=0..31.
    Finalize: PE-transpose acc_sb 128-q blocks, out = ot/(8*rowsum).

Scheduling: chunk-granular input DMA; PE warms up on wt; proj chains,
V transposes and finalizes are interleaved into the slot stream as PE
filler; AV of slot i-1 is emitted after scores of slot i so the PE
never sits directly behind the ACT exp.
"""

import sys

import numpy as np

for _p in ("/opt/trn_rl_repo",):
    if _p not in sys.path:
        sys.path.insert(0, _p)

import concourse.mybir as mybir  # noqa: E402
import concourse.tile as tile  # noqa: E402
from concourse import bacc  # noqa: E402
from concourse.bass_utils import run_bass_kernel_spmd  # noqa: E402
from concourse.masks import make_identity  # noqa: E402

B, S, D, DH = 4, 4096, 768, 64
QN = S // 2          # queries per core
NSB = 4              # 1024-wide super-blocks of s
SBW = 1024
NKT = 32             # 128-wide key tiles
HALF = 1024          # q-half for the slot loop
EXP_BIAS = -40.0     # global score offset (softmax-invariant), fp32 headroom
MMW = 512            # matmul rhs width (hard ISA cap: N<=512 per matmul)

F32 = mybir.dt.float32
F16 = mybir.dt.float16
BF16 = mybir.dt.bfloat16

_NC_CACHE = None
LAST_RESULTS = None


def _build():
    nc = bacc.Bacc(num_devices=8)
    xt_d = nc.dram_tensor("xt", [D, S], F16, kind="ExternalInput")
    # wt cols: 6 contraction chunks x [K|V (128) | Q (64)]
    wt_d = nc.dram_tensor("wt", [128, 6 * 192], F16, kind="ExternalInput")
    out_d = nc.dram_tensor("out", [QN, DH], F32, kind="ExternalOutput")

    with tile.TileContext(nc) as tc:
        with (
            tc.tile_pool(name="big", bufs=1) as big,
            tc.tile_pool(name="psmm", bufs=3, space="PSUM") as psmm,
            tc.tile_pool(name="psacc", bufs=1, space="PSUM") as psacc,
            tc.tile_pool(name="pt", bufs=6) as ptp,
            tc.tile_pool(name="small", bufs=4) as small,
        ):
            xt_tiles = [
                big.tile([128, 6, SBW], F16, tag=f"xt{sb}", name=f"xt{sb}")
                for sb in range(NSB)
            ]
            wt_sb = big.tile([128, 6 * 192], F16)
            kt = big.tile([64, S], F16)
            qt = big.tile([64, QN], F16)
            vt = big.tile([64, S], BF16)
            v_sb = big.tile([128, NKT, 80], BF16)  # [...,0:64]=V, 64=ones
            acc_sb = big.tile([65, QN], F32)
            osb = big.tile([128, 16, DH], F32)
            ident = big.tile([128, 128], F32)
            identb = big.tile([128, 128], BF16)
            ebias = big.tile([128, 1], F32)
            escr = big.tile([128, 1], F32)

            # ---- input DMA split across issue queues: SP (hwdge) takes wt
            # + sb0 chunk-granular; ACT (hwdge) sb1; Pool (swdge on the idle
            # q7s) sb2+sb3. SP alone needs ~610ns per issue, serializing the
            # whole input stream ~15us; the split gets every sb moving early.
            def _xt_src(sb):
                return xt_d[:, sb * SBW:(sb + 1) * SBW].rearrange(
                    "(k p) s -> p k s", p=128
                )

            # wt rides the ACT hwdge queue so SP's first issue is already
            # sb0 chunk 0; all xt stays on SP IN CONSUMPTION ORDER (a
            # multi-queue split lets sb2/3 steal HBM bandwidth from the
            # sb0/sb1 transfers the PE needs first).
            nc.scalar.dma_start(out=wt_sb[:], in_=wt_d[:])
            for k in range(6):
                nc.sync.dma_start(
                    out=xt_tiles[0][:, k:k + 1, :], in_=_xt_src(0)[:, k:k + 1, :]
                )
            for sb in (1, 2, 3):
                for k0 in range(0, 6, 2):
                    nc.sync.dma_start(
                        out=xt_tiles[sb][:, k0:k0 + 2, :],
                        in_=_xt_src(sb)[:, k0:k0 + 2, :],
                    )

            # ---- cheap setup off the PE
            wsrc = big.tile([128, 512], BF16)
            nc.vector.memset(wsrc[:], 0.0)
            nc.vector.memset(ebias[:], EXP_BIAS)
            nc.vector.memset(v_sb[:, :, 64:65], 1.0)
            make_identity(nc, ident[:])
            make_identity(nc, identb[:])
            # prime the ACT exp table set during the DMA window
            nc.scalar.activation(
                out=escr[:], in_=ebias[:],
                func=mybir.ActivationFunctionType.Exp, bias=ebias[:],
            )

            # ---- PE warmup, DMA-independent (opens the HAM clock-gate; a
            # ramp gap resets the 3us warm-up window, so bridge until sb0
            # chunks flow)
            for _w in range(6):
                wps = psmm.tile([128, HALF], F32, tag="mm")
                nc.tensor.matmul(
                    wps[:, 0:512], wsrc[:, 0:128], wsrc[:],
                    start=True, stop=True,
                )

            # ---- emission helpers --------------------------------------
            kv_state = {}

            def emit_kv(sb, k0, k1):
                """Chunks k0:k1 of sb's K/V projection chain."""
                if k0 == 0:
                    kv_state[sb] = psmm.tile(
                        [128, HALF], F32, tag="mm", name=f"kv_ps{sb}"
                    )
                ps = kv_state[sb]
                for k in range(k0, k1):
                    for c in range(HALF // MMW):
                        csl = slice(c * MMW, (c + 1) * MMW)
                        nc.tensor.matmul(
                            ps[:, csl],
                            wt_sb[:, k * 192:k * 192 + 128],
                            xt_tiles[sb][:, k, c * MMW:(c + 1) * MMW],
                            start=(k == 0), stop=(k == 5),
                        )
                if k1 == 6:
                    s0 = sb * SBW
                    nc.vector.tensor_copy(kt[:, s0:s0 + 512], ps[0:64, 0:512])
                    nc.vector.tensor_copy(
                        kt[:, s0 + 512:s0 + SBW], ps[0:64, 512:SBW]
                    )
                    nc.vector.tensor_copy(
                        vt[:, s0:s0 + 512], ps[64:128, 0:512]
                    )
                    nc.vector.tensor_copy(
                        vt[:, s0 + 512:s0 + SBW], ps[64:128, 512:SBW]
                    )

            def emit_q(sb, k0, k1):
                """Chunks k0:k1 of sb's Q projection chain (sb 0 or 1)."""
                if k0 == 0:
                    kv_state[("q", sb)] = psmm.tile(
                        [128, HALF], F32, tag="mm", name=f"q_ps{sb}"
                    )
                ps = kv_state[("q", sb)]
                for k in range(k0, k1):
                    for c in range(HALF // MMW):
                        csl = slice(c * MMW, (c + 1) * MMW)
                        nc.tensor.matmul(
                            ps[0:64, csl],
                            wt_sb[:, k * 192 + 128:(k + 1) * 192],
                            xt_tiles[sb][:, k, c * MMW:(c + 1) * MMW],
                            start=(k == 0), stop=(k == 5),
                        )
                if k1 == 6:
                    nc.vector.tensor_copy(
                        qt[:, sb * SBW:(sb + 1) * SBW], ps[0:64, :]
                    )
                    kv_state.pop(("q", sb))

            def emit_vtrans(sb, i0, i1):
                """V natural tiles i0:i1 (of 8) for sb via PE transpose into
                the spent kv_ps tile (bf16 view; WAR on the kt/vt copies)."""
                ps16 = kv_state[sb][:].bitcast(BF16)
                for i in range(i0, i1):
                    t = sb * 8 + i
                    tsl = slice(i * 64, (i + 1) * 64)
                    nc.tensor.transpose(
                        ps16[:, tsl],
                        vt[:, t * 128:(t + 1) * 128],
                        identb[0:64, 0:64],
                    )
                    nc.vector.tensor_copy(v_sb[:, t, 0:64], ps16[:, tsl])
                if i1 == 8:
                    kv_state.pop(sb)

            ats = {}
            pts = {}
            accs = {}

            def emit_scores(h, t):
                at = psmm.tile([128, HALF], F32, tag="mm")
                for c in range(HALF // MMW):
                    csl = slice(c * MMW, (c + 1) * MMW)
                    qsl = slice(h * HALF + c * MMW, h * HALF + (c + 1) * MMW)
                    nc.tensor.matmul(
                        at[:, csl],
                        kt[:, t * 128:(t + 1) * 128],
                        qt[:, qsl],
                        start=True, stop=True,
                    )
                pt = ptp.tile([128, HALF], BF16, tag="pt")
                nc.scalar.activation(
                    out=pt[:], in_=at[:],
                    func=mybir.ActivationFunctionType.Exp, bias=ebias[:],
                )
                pts[(h, t)] = pt

            def emit_av(h, t):
                pt = pts.pop((h, t))
                acc = accs[h]
                for c in range(HALF // MMW):
                    csl = slice(c * MMW, (c + 1) * MMW)
                    nc.tensor.matmul(
                        acc[:, csl],
                        v_sb[:, t, 0:65],
                        pt[:, csl],
                        start=(t == 0), stop=(t == NKT - 1),
                        skip_group_check=True,
                    )

            def emit_out_dma(oc):
                nc.sync.dma_start(
                    out=out_d[:].rearrange("(t p) d -> p t d", p=128)[
                        :, 4 * oc:4 * (oc + 1), :],
                    in_=osb[:, 4 * oc:4 * (oc + 1), :],
                )

            def emit_fin(gblks):
                """Batch: transpose each 128-q block into one psum tile, one
                batched reciprocal, then per-block scale (avoids the PE-DVE
                ping-pong that stalled the tail)."""
                n = len(gblks)
                ot = psmm.tile([128, HALF], F32, tag="mm")
                for j, g in enumerate(gblks):
                    nc.tensor.transpose(
                        ot[:, 128 * j:128 * j + 65],
                        acc_sb[:, g * 128:(g + 1) * 128],
                        ident[0:65, 0:65],
                    )
                r = small.tile([128, 4], F32, tag="r")
                nc.vector.reciprocal(
                    r[:, 0:n],
                    ot[:].rearrange("p (j c) -> p j c", c=128)[:, 0:n, 64:65],
                )
                for j, g in enumerate(gblks):
                    nc.vector.tensor_scalar(
                        osb[:, g, :], ot[:, 128 * j:128 * j + 64], r[:, j:j + 1],
                        0.125,
                        op0=mybir.AluOpType.mult, op1=mybir.AluOpType.mult,
                    )

            # ---- pre-slot ramp: sb0 proj (kv then q), then V tiles 0:8 ----
            emit_kv(0, 0, 6)
            emit_q(0, 0, 6)
            emit_vtrans(0, 0, 8)
            accs[0] = psacc.tile([65, HALF], F32, tag="acc", name="acc0")

            # ---- slot stream -------------------------------------------
            # filler[i]: PE work emitted right after scores of slot i.
            # kv_sb must land before its slots (kt t: sb1 -> slots 8.., etc);
            # Vtrans sb before its AV slots; q1 before slot 32; all proj
            # windows sit in h0 so h1 slots have a free psmm buf for fins.
            filler = {
                0: [("kv", 1, 0, 2)], 1: [("kv", 1, 2, 4)], 2: [("kv", 1, 4, 6)],
                3: [("vt", 1, 0, 2)], 4: [("vt", 1, 2, 4)],
                5: [("vt", 1, 4, 6)], 6: [("vt", 1, 6, 8)],
                7: [("kv", 2, 0, 2)], 8: [("kv", 2, 2, 4)], 9: [("kv", 2, 4, 6)],
                10: [("vt", 2, 0, 2)], 11: [("vt", 2, 2, 4)],
                12: [("vt", 2, 4, 6)], 13: [("vt", 2, 6, 8)],
                14: [("kv", 3, 0, 2)], 15: [("kv", 3, 2, 4)], 16: [("kv", 3, 4, 6)],
                17: [("vt", 3, 0, 2)], 18: [("vt", 3, 2, 4)],
                19: [("vt", 3, 4, 6)], 20: [("vt", 3, 6, 8)],
                21: [("q", 1, 0, 2)], 22: [("q", 1, 2, 4)], 23: [("q", 1, 4, 6)],
                # h1: finalize h0 blocks two per slot-pair; out DMAs staggered
                34: [("fin", (0, 1))], 36: [("fin", (2, 3))],
                38: [("fin", (4, 5))], 39: [("odma", 0)],
                40: [("fin", (6, 7))], 41: [("odma", 1)],
            }

            slots = [(0, t) for t in range(NKT)] + [(1, t) for t in range(NKT)]
            for i, (h, t) in enumerate(slots):
                emit_scores(h, t)
                for f in filler.get(i, ()):
                    if f[0] == "kv":
                        emit_kv(f[1], f[2], f[3])
                    elif f[0] == "q":
                        emit_q(f[1], f[2], f[3])
                    elif f[0] == "vt":
                        emit_vtrans(f[1], f[2], f[3])
                    elif f[0] == "fin":
                        emit_fin(f[1])
                    elif f[0] == "odma":
                        emit_out_dma(f[1])
                if i > 0:
                    ph, pt_ = slots[i - 1]
                    emit_av(ph, pt_)
                    if (ph, pt_) == (0, NKT - 1):
                        # h0 accumulation complete: stage and swap acc
                        nc.vector.tensor_copy(acc_sb[:, 0:512], accs[0][:, 0:512])
                        nc.vector.tensor_copy(
                            acc_sb[:, 512:HALF], accs[0][:, 512:HALF]
                        )
                        accs[1] = psacc.tile(
                            [65, HALF], F32, tag="acc", name="acc1"
                        )

            # ---- tail: last AV split so each 512-q chunk stages early ----
            fpt = pts.pop((1, NKT - 1))
            for c in range(2):
                csl = slice(c * 512, (c + 1) * 512)
                nc.tensor.matmul(
                    accs[1][:, csl], v_sb[:, NKT - 1, 0:65], fpt[:, csl],
                    start=False, stop=True, skip_group_check=True,
                )
                nc.vector.tensor_copy(
                    acc_sb[:, HALF + c * 512:HALF + (c + 1) * 512],
                    accs[1][:, csl],
                )
                emit_fin(tuple(range(8 + 4 * c, 12 + 4 * c)))
                emit_out_dma(2 + c)

    nc.finalize()
    return nc


def _get_nc():
    global _NC_CACHE
    if _NC_CACHE is None:
        _NC_CACHE = _build()
    return _NC_CACHE


def kernel(x, W, _trace=False):
    global LAST_RESULTS
    x = np.ascontiguousarray(np.asarray(x), dtype=np.float32)
    W = np.ascontiguousarray(np.asarray(W), dtype=np.float32)
    assert x.shape == (B, S, D) and W.shape == (3 * DH, D)

    # wt cols per contraction chunk k: [K rows | V rows | Q rows]
    wtf = np.concatenate([W[DH:2 * DH], W[2 * DH:], W[:DH]], axis=0).T
    wt = np.ascontiguousarray(
        wtf.reshape(6, 128, 192).transpose(1, 0, 2).reshape(128, 6 * 192)
    ).astype(np.float16)

    in_maps = []
    for c in range(8):
        b, qh = divmod(c, 2)
        xtb = x[b].T.astype(np.float16)  # [768, 4096]
        if qh:
            xtc = np.ascontiguousarray(
                np.concatenate([xtb[:, QN:], xtb[:, :QN]], axis=1)
            )
        else:
            xtc = np.ascontiguousarray(xtb)
        in_maps.append({"xt": xtc, "wt": wt})

    nc = _get_nc()
    res = run_bass_kernel_spmd(nc, in_maps, list(range(8)), trace=_trace)
    LAST_RESULTS = res

    out = np.empty((B, S, DH), np.float32)
    for c in range(8):
        b, qh = divmod(c, 2)
        out[b, qh * QN:(qh + 1) * QN] = res.results[c]["out"]
    return out
